# revision 6
# baseline (speedup 1.0000x reference)
"""BiLSTM classifier Trainium2 kernel.

Data-parallel over batch across 8 NeuronCores: each core runs the full
BiLSTM (fwd LSTM, bwd LSTM, 2nd LSTM, classifier head) for its 32-row
batch shard, with replicated weights. All matmuls in bf16 (fp32 PSUM
accumulate); verified numerics: max rel err vs fp32 reference ~3e-4.

Layout notes (per core, B=32 local batch):
- Recurrent matmuls keep the batch as the PE stationary operand
  (lhsT = h^T [K-tile, 32]) and stream the weights as the moving
  operand, so per-step PE time ~ (#weight columns) regardless of M.
- Gates are reordered [i|f|o|g] (host-side weight row permutation).
- h is transposed each step via PE-transpose (identity matmul) to feed
  the next step's stationary operand; c stays in batch-major layout.
- X := x @ W_ih^T + b precomputed for all timesteps (fully parallel),
  added to the hh-matmul PSUM via identity-matmul accumulation (layer 1)
  or DVE add (layer 2).
"""

import sys

sys.path.insert(0, "/opt/trn_rl_repo")

import numpy as np
import ml_dtypes

import concourse.bass as bass
import concourse.mybir as mybir
import concourse.tile as tile
from concourse import bacc
from concourse.bass_utils import run_bass_kernel_spmd

AF = mybir.ActivationFunctionType
BF16 = mybir.dt.bfloat16
F32 = mybir.dt.float32

B, D, H = 256, 256, 512
H2 = 2 * H          # 1024 second-layer hidden
L = 2
NCORES = 8
BL = B // NCORES    # 32 local batch


def _build_nc(T: int):
    nc = bacc.Bacc(None, target_bir_lowering=False)

    # ---- DRAM inputs (per-core) ----
    xT = nc.dram_tensor("xT", [D, T, BL], BF16, kind="ExternalInput")
    wf_ih = nc.dram_tensor("wf_ih", [D, 4 * H], BF16, kind="ExternalInput")
    wf_hh = nc.dram_tensor("wf_hh", [H, 4 * H], BF16, kind="ExternalInput")
    wb_ih = nc.dram_tensor("wb_ih", [D, 4 * H], BF16, kind="ExternalInput")
    wb_hh = nc.dram_tensor("wb_hh", [H, 4 * H], BF16, kind="ExternalInput")
    ws_ih = nc.dram_tensor("ws_ih", [H2, 4 * H2], BF16, kind="ExternalInput")
    ws_hh = nc.dram_tensor("ws_hh", [H2, 4 * H2], BF16, kind="ExternalInput")
    bf_r = nc.dram_tensor("bf_r", [1, 4 * H], BF16, kind="ExternalInput")
    bb_r = nc.dram_tensor("bb_r", [1, 4 * H], BF16, kind="ExternalInput")
    bs_r = nc.dram_tensor("bs_r", [1, 4 * H2], BF16, kind="ExternalInput")
    wl = nc.dram_tensor("wl", [H2, L], BF16, kind="ExternalInput")
    bl_r = nc.dram_tensor("bl_r", [1, L], BF16, kind="ExternalInput")
    ones_r = nc.dram_tensor("ones_r", [1, 128], BF16, kind="ExternalInput")
    id32 = nc.dram_tensor("id32", [32, 32], BF16, kind="ExternalInput")
    out = nc.dram_tensor("out", [BL, L], F32, kind="ExternalOutput")

    with tile.TileContext(nc) as tc:
        _emit(nc, tc, T, locals())
    nc.compile()
    return nc


def _emit(nc, tc, T, t_):
    xT, out = t_["xT"], t_["out"]
    MT = T * BL // 128  # m-tiles of 128 rows (4 timesteps x 32 batch)

    from contextlib import ExitStack

    with ExitStack() as ctx:
        ec = ctx.enter_context

        dram = ec(tc.tile_pool(name="dram", bufs=1, space="DRAM"))
        const = ec(tc.tile_pool(name="const", bufs=1))
        state = ec(tc.tile_pool(name="state", bufs=1))

        Xf = dram.tile([T, BL, 4 * H], BF16, tag="Xf", name="Xf")
        Xb = dram.tile([T, BL, 4 * H], BF16, tag="Xb", name="Xb")
        Xs = dram.tile([T, BL, 4 * H2], BF16, tag="Xs", name="Xs")
        fT = dram.tile([4, 128, T, 32], BF16, tag="fT", name="fT")
        bT = dram.tile([4, 128, T, 32], BF16, tag="bT", name="bT")

        ones_t = const.tile([1, 128], BF16)
        nc.sync.dma_start(out=ones_t, in_=t_["ones_r"][:, :])
        id32_t = const.tile([32, 32], BF16)
        nc.sync.dma_start(out=id32_t, in_=t_["id32"][:, :])

        # ---- per-layer-1-direction persistent state ----
        hT = {}
        cst = {}
        for d in ("f", "b"):
            hT[d] = state.tile([128, 4, 32], BF16, tag=f"hT_{d}", name=f"hT_{d}")
            nc.vector.memset(hT[d], 0.0)
            cst[d] = state.tile([BL, H], F32, tag=f"c_{d}", name=f"c_{d}")
            nc.vector.memset(cst[d], 0.0)
        h2T = state.tile([128, 8, 32], BF16, tag="h2T", name="h2T")
        nc.vector.memset(h2T, 0.0)
        c2 = state.tile([BL, H2], F32, tag="c2", name="c2")
        nc.vector.memset(c2, 0.0)

        # =========================================================
        # Phase ih: X{f,b}[t] = x_t @ W_ih^T + b  for all t
        # =========================================================
        with tc.tile_pool(name="ihw", bufs=1) as ihw, \
             tc.tile_pool(name="ihx", bufs=3) as ihx, \
             tc.tile_pool(name="ihp", bufs=2, space="PSUM") as ihp, \
             tc.tile_pool(name="iho", bufs=3) as iho:
            w_sb = {}
            b_sb = {}
            for d, wdram, bdram in (("f", t_["wf_ih"], t_["bf_r"]),
                                    ("b", t_["wb_ih"], t_["bb_r"])):
                w_sb[d] = ihw.tile([128, 2, 4 * H], BF16, tag=f"wih_{d}", name=f"wih_{d}")
                nc.sync.dma_start(out=w_sb[d], in_=wdram.rearrange("(k p) n -> p k n", p=128))
                b_sb[d] = ihw.tile([1, 4 * H], BF16, tag=f"bih_{d}", name=f"bih_{d}")
                nc.sync.dma_start(out=b_sb[d], in_=bdram[:, :])

            for d, Xd in (("f", Xf), ("b", Xb)):
                for mt in range(MT):
                    t0 = mt * 4
                    xt = ihx.tile([128, 2, 4, 32], BF16, tag="xt", name="xt")
                    nc.sync.dma_start(
                        out=xt,
                        in_=xT.rearrange("(k p) t b -> p k t b", p=128)[:, :, t0:t0 + 4, :],
                    )
                    ps = ihp.tile([128, 4 * H], F32, tag="ps", name="ps")
                    for nh in range(4):
                        nsl = slice(nh * 512, (nh + 1) * 512)
                        for kt in range(2):
                            nc.tensor.matmul(
                                ps[:, nsl],
                                xt[:, kt].rearrange("p t b -> p (t b)"),
                                w_sb[d][:, kt, nsl],
                                start=(kt == 0), stop=False,
                            )
                        nc.tensor.matmul(
                            ps[:, nsl], ones_t[:, :128], b_sb[d][:, nsl],
                            start=False, stop=True,
                        )
                    ot = iho.tile([128, 4 * H], BF16, tag="ot", name="ot")
                    nc.vector.tensor_copy(ot, ps)
                    nc.sync.dma_start(
                        out=Xd[t0:t0 + 4].rearrange("t b n -> (t b) n"),
                        in_=ot,
                    )

        # =========================================================
        # Phase A: fwd + bwd LSTM recurrences (interleaved)
        # =========================================================
        with tc.tile_pool(name="aw", bufs=1) as aw, \
             tc.tile_pool(name="ax", bufs=3) as ax, \
             tc.tile_pool(name="ag", bufs=2, space="PSUM") as ag, \
             tc.tile_pool(name="atr", bufs=1, space="PSUM") as atr, \
             tc.tile_pool(name="aact", bufs=2) as aact:
            whh_sb = {}
            for d, wdram in (("f", t_["wf_hh"]), ("b", t_["wb_hh"])):
                whh_sb[d] = aw.tile([128, 4, 4 * H], BF16, tag=f"whh_{d}", name=f"whh_{d}")
                nc.sync.dma_start(out=whh_sb[d], in_=wdram.rearrange("(k p) n -> p k n", p=128))

            def l1_step(d, s):
                Xd = Xf if d == "f" else Xb
                x_idx = s if d == "f" else T - 1 - s
                outT = fT if d == "f" else bT
                xt = ax.tile([BL, 4 * H], BF16, tag=f"xa_{d}", name=f"xa_{d}")
                nc.sync.dma_start(out=xt, in_=Xd[x_idx])
                # gates quarters in [i|f|o|g] order, each [BL, 512] in PSUM
                q = []
                for qi in range(4):
                    ps = ag.tile([BL, H], F32, tag=f"g_{d}", name=f"g{qi}_{d}")
                    nsl = slice(qi * H, (qi + 1) * H)
                    for kt in range(4):
                        nc.tensor.matmul(
                            ps, hT[d][:, kt], whh_sb[d][:, kt, nsl],
                            start=(kt == 0), stop=False,
                        )
                    nc.tensor.matmul(
                        ps, id32_t, xt[:, nsl], start=False, stop=True,
                    )
                    q.append(ps)
                si = aact.tile([BL, H], F32, tag=f"si_{d}", name=f"si_{d}")
                nc.scalar.activation(si, q[0], AF.Sigmoid)
                sf = aact.tile([BL, H], F32, tag=f"sf_{d}", name=f"sf_{d}")
                nc.scalar.activation(sf, q[1], AF.Sigmoid)
                so = aact.tile([BL, H], F32, tag=f"so_{d}", name=f"so_{d}")
                nc.scalar.activation(so, q[2], AF.Sigmoid)
                tg = aact.tile([BL, H], F32, tag=f"tg_{d}", name=f"tg_{d}")
                nc.scalar.activation(tg, q[3], AF.Tanh)

                v = aact.tile([BL, H], F32, tag=f"v_{d}", name=f"v_{d}")
                nc.vector.tensor_mul(v, sf, cst[d])          # f * c
                u = aact.tile([BL, H], F32, tag=f"u_{d}", name=f"u_{d}")
                nc.gpsimd.tensor_mul(u, si, tg)              # i * g
                nc.vector.tensor_add(cst[d], u, v)           # c = u + v
                tc_ = aact.tile([BL, H], F32, tag=f"tc_{d}", name=f"tc_{d}")
                nc.scalar.activation(tc_, cst[d], AF.Tanh)
                h = aact.tile([BL, H], BF16, tag=f"h_{d}", name=f"h_{d}")
                nc.vector.tensor_mul(h, so, tc_)             # h = o * tanh(c)

                ptr = atr.tile([128, 4, 32], BF16, tag=f"tr_{d}", name=f"tr_{d}")
                for kt in range(4):
                    nc.tensor.transpose(
                        ptr[:, kt], h[:, kt * 128:(kt + 1) * 128], id32_t,
                    )
                nc.vector.tensor_copy(hT[d], ptr)
                nc.sync.dma_start(
                    out=outT[:, :, s, :].rearrange("k p b -> p k b"),
                    in_=hT[d],
                )

            for s in range(T):
                l1_step("f", s)
                l1_step("b", s)

        # =========================================================
        # Phase Xs: Xs[t] = combined_t @ Ws_ih^T + bs
        # =========================================================
        with tc.tile_pool(name="sw", bufs=1) as sw, \
             tc.tile_pool(name="sk", bufs=3) as sk, \
             tc.tile_pool(name="sp", bufs=2, space="PSUM") as sp, \
             tc.tile_pool(name="so_", bufs=3) as so_:
            wsih_sb = sw.tile([128, 8, 4 * H2], BF16, tag="wsih", name="wsih")
            nc.sync.dma_start(out=wsih_sb, in_=t_["ws_ih"].rearrange("(k p) n -> p k n", p=128))
            bs_sb = sw.tile([1, 4 * H2], BF16, tag="bs", name="bs")
            nc.sync.dma_start(out=bs_sb, in_=t_["bs_r"][:, :])

            for mt in range(MT):
                t0 = mt * 4
                ck = sk.tile([128, 8, 4, 32], BF16, tag="ck", name="ck")
                for kt in range(8):
                    src = fT if kt < 4 else bT
                    nc.sync.dma_start(out=ck[:, kt], in_=src[kt % 4, :, t0:t0 + 4, :])
                for half in range(2):
                    ps = sp.tile([128, 2 * H2], F32, tag="ps", name="ps")
                    for nh in range(4):
                        nsl_p = slice(nh * 512, (nh + 1) * 512)
                        nsl_w = slice(half * 2048 + nh * 512, half * 2048 + (nh + 1) * 512)
                        for kt in range(8):
                            nc.tensor.matmul(
                                ps[:, nsl_p],
                                ck[:, kt].rearrange("p t b -> p (t b)"),
                                wsih_sb[:, kt, nsl_w],
                                start=(kt == 0), stop=False,
                            )
                        nc.tensor.matmul(
                            ps[:, nsl_p], ones_t[:, :128], bs_sb[:, nsl_w],
                            start=False, stop=True,
                        )
                    ot = so_.tile([128, 2 * H2], BF16, tag="ot", name="ot")
                    nc.vector.tensor_copy(ot, ps)
                    nc.sync.dma_start(
                        out=Xs[t0:t0 + 4, :, half * 2048:(half + 1) * 2048]
                            .rearrange("t b n -> (t b) n"),
                        in_=ot,
                    )

        # =========================================================
        # Phase B: second LSTM over combined, keep final h
        # =========================================================
        with tc.tile_pool(name="bw", bufs=1) as bw, \
             tc.tile_pool(name="bx", bufs=3) as bx, \
             tc.tile_pool(name="bg", bufs=3, space="PSUM") as bg, \
             tc.tile_pool(name="btr", bufs=1, space="PSUM") as btr, \
             tc.tile_pool(name="bact", bufs=1) as bact:
            wshh_sb = bw.tile([128, 8, 4 * H2], BF16, tag="wshh", name="wshh")
            nc.sync.dma_start(out=wshh_sb, in_=t_["ws_hh"].rearrange("(k p) n -> p k n", p=128))

            for s in range(T):
                # quarters [i|f|o|g], each [BL, 1024]
                gq = []
                for qi in range(4):
                    xt = bx.tile([BL, H2], BF16, tag="xb", name=f"xb{qi}")
                    nc.sync.dma_start(out=xt, in_=Xs[s, :, qi * H2:(qi + 1) * H2])
                    ps = bg.tile([BL, H2], F32, tag="bg", name=f"bg{qi}")
                    for nh in range(2):
                        psl = slice(nh * 512, (nh + 1) * 512)
                        nsl = slice(qi * H2 + nh * 512, qi * H2 + (nh + 1) * 512)
                        for kt in range(8):
                            nc.tensor.matmul(
                                ps[:, psl], h2T[:, kt], wshh_sb[:, kt, nsl],
                                start=(kt == 0), stop=(kt == 7),
                            )
                    g = bact.tile([BL, H2], F32, tag=f"gb{qi}", name=f"gb{qi}")
                    nc.vector.tensor_add(g, ps, xt)
                    gq.append(g)
                si = bact.tile([BL, H2], F32, tag="si2", name="si2")
                nc.scalar.activation(si, gq[0], AF.Sigmoid)
                sf = bact.tile([BL, H2], F32, tag="sf2", name="sf2")
                nc.scalar.activation(sf, gq[1], AF.Sigmoid)
                so2 = bact.tile([BL, H2], F32, tag="so2", name="so2")
                nc.scalar.activation(so2, gq[2], AF.Sigmoid)
                tg = bact.tile([BL, H2], F32, tag="tg2", name="tg2")
                nc.scalar.activation(tg, gq[3], AF.Tanh)

                v = bact.tile([BL, H2], F32, tag="v2", name="v2")
                nc.vector.tensor_mul(v, sf, c2)
                u = bact.tile([BL, H2], F32, tag="u2", name="u2")
                nc.gpsimd.tensor_mul(u, si, tg)
                nc.vector.tensor_add(c2, u, v)
                tc2 = bact.tile([BL, H2], F32, tag="tc2", name="tc2")
                nc.scalar.activation(tc2, c2, AF.Tanh)
                h2 = bact.tile([BL, H2], BF16, tag="h2", name="h2")
                nc.vector.tensor_mul(h2, so2, tc2)

                ptr = btr.tile([128, 8, 32], BF16, tag="tr2", name="tr2")
                for kt in range(8):
                    nc.tensor.transpose(
                        ptr[:, kt], h2[:, kt * 128:(kt + 1) * 128], id32_t,
                    )
                nc.vector.tensor_copy(h2T, ptr)

            # ---- classifier: out = sigmoid(h2 @ Wl^T + bl) ----
            wl_sb = bw.tile([128, 8, L], BF16, tag="wl", name="wl")
            nc.sync.dma_start(out=wl_sb, in_=t_["wl"].rearrange("(k p) n -> p k n", p=128))
            bl_sb = bw.tile([1, L], BF16, tag="bl", name="bl")
            nc.sync.dma_start(out=bl_sb, in_=t_["bl_r"][:, :])
            ps_o = btr.tile([BL, L], F32, tag="ps_o", name="ps_o")
            for kt in range(8):
                nc.tensor.matmul(
                    ps_o, h2T[:, kt], wl_sb[:, kt],
                    start=(kt == 0), stop=False,
                )
            nc.tensor.matmul(ps_o, ones_t[:, :BL], bl_sb, start=False, stop=True)
            o_sb = bact.tile([BL, L], F32, tag="o_sb", name="o_sb")
            nc.scalar.activation(o_sb, ps_o, AF.Sigmoid)
            nc.sync.dma_start(out=out[:, :], in_=o_sb)


_NC_CACHE = {}


def _get_nc(T):
    if T not in _NC_CACHE:
        _NC_CACHE[T] = _build_nc(T)
    return _NC_CACHE[T]


def _bf16(a):
    return np.ascontiguousarray(a).astype(ml_dtypes.bfloat16)


def _prep_weights(Wf_ih, Wf_hh, bf, Wb_ih, Wb_hh, bb, Ws_ih, Ws_hh, bs, Wl, bl):
    # gate reorder [i|f|g|o] -> [i|f|o|g]
    r1 = np.r_[0:H, H:2 * H, 3 * H:4 * H, 2 * H:3 * H]
    r2 = np.r_[0:H2, H2:2 * H2, 3 * H2:4 * H2, 2 * H2:3 * H2]
    m = {
        "wf_ih": _bf16(Wf_ih[r1].T),
        "wf_hh": _bf16(Wf_hh[r1].T),
        "wb_ih": _bf16(Wb_ih[r1].T),
        "wb_hh": _bf16(Wb_hh[r1].T),
        "ws_ih": _bf16(Ws_ih[r2].T),
        "ws_hh": _bf16(Ws_hh[r2].T),
        "bf_r": _bf16(bf[r1][None, :]),
        "bb_r": _bf16(bb[r1][None, :]),
        "bs_r": _bf16(bs[r2][None, :]),
        "wl": _bf16(Wl.T),
        "bl_r": _bf16(bl[None, :]),
        "ones_r": _bf16(np.ones((1, 128), np.float32)),
        "id32": _bf16(np.eye(32, dtype=np.float32)),
    }
    return m


def kernel(x, Wf_ih, Wf_hh, bf, Wb_ih, Wb_hh, bb, Ws_ih, Ws_hh, bs, Wl, bl):
    x = np.asarray(x, dtype=np.float32)
    T = x.shape[1]
    nc = _get_nc(T)
    wmap = _prep_weights(
        np.asarray(Wf_ih), np.asarray(Wf_hh), np.asarray(bf),
        np.asarray(Wb_ih), np.asarray(Wb_hh), np.asarray(bb),
        np.asarray(Ws_ih), np.asarray(Ws_hh), np.asarray(bs),
        np.asarray(Wl), np.asarray(bl),
    )
    in_maps = []
    for c in range(NCORES):
        xc = x[c * BL:(c + 1) * BL]            # [BL, T, D]
        m = dict(wmap)
        m["xT"] = _bf16(xc.transpose(2, 1, 0))  # [D, T, BL]
        in_maps.append(m)
    res = run_bass_kernel_spmd(nc, in_maps, list(range(NCORES)))
    return np.concatenate([res.results[c]["out"] for c in range(NCORES)], axis=0)


if __name__ == "__main__":
    rng = np.random.default_rng(0)
    T = int(sys.argv[1]) if len(sys.argv) > 1 else 8
    ins = {
        "x": rng.standard_normal((B, T, D), dtype=np.float32),
        "Wf_ih": rng.standard_normal((4 * H, D), dtype=np.float32) * 0.05,
        "Wf_hh": rng.standard_normal((4 * H, H), dtype=np.float32) * 0.04,
        "bf": np.zeros(4 * H, np.float32),
        "Wb_ih": rng.standard_normal((4 * H, D), dtype=np.float32) * 0.05,
        "Wb_hh": rng.standard_normal((4 * H, H), dtype=np.float32) * 0.04,
        "bb": np.zeros(4 * H, np.float32),
        "Ws_ih": rng.standard_normal((4 * H2, H2), dtype=np.float32) * 0.03,
        "Ws_hh": rng.standard_normal((4 * H2, H2), dtype=np.float32) * 0.03,
        "bs": np.zeros(4 * H2, np.float32),
        "Wl": rng.standard_normal((L, H2), dtype=np.float32) * 0.04,
        "bl": np.zeros(L, np.float32),
    }
    got = kernel(**ins)

    def sigmoid(z):
        return 1.0 / (1.0 + np.exp(-z))

    def scan(xs, Wih, Whh, bvec):
        Tn, Bn, _ = xs.shape
        Hh = Whh.shape[1]
        h = np.zeros((Bn, Hh), np.float32)
        c = np.zeros((Bn, Hh), np.float32)
        hs = []
        for t in range(Tn):
            g = xs[t] @ Wih.T + h @ Whh.T + bvec
            i, f, gg, o = np.split(g, 4, axis=-1)
            c = sigmoid(f) * c + sigmoid(i) * np.tanh(gg)
            h = sigmoid(o) * np.tanh(c)
            hs.append(h.copy())
        return np.stack(hs), h

    xs = np.swapaxes(ins["x"], 0, 1)
    fseq, _ = scan(xs, ins["Wf_ih"], ins["Wf_hh"], ins["bf"])
    bseq, _ = scan(xs[::-1], ins["Wb_ih"], ins["Wb_hh"], ins["bb"])
    comb = np.concatenate([fseq, bseq], -1)
    _, hs = scan(comb, ins["Ws_ih"], ins["Ws_hh"], ins["bs"])
    ref = sigmoid(hs @ ins["Wl"].T + ins["bl"])
    rel = np.abs(got - ref) / np.maximum(np.abs(ref), 1e-6)
    print(f"T={T}: max rel {rel.max():.3e}  mean rel {rel.mean():.3e}")
